# revision 30
# baseline (speedup 1.0000x reference)
"""Fused transformer block (QKV proj + attention + FFN + 2x LayerNorm) on 8
Trainium2 NeuronCores.

Sharding: batch (B=2) across two 4-core groups; within a group, tensor
parallel over heads (4 heads / core) for projections+attention, then a
2-chunk AllToAll switches to row sharding for the FFN/LayerNorm tail.

v3: fp8e4m3 DoubleRow matmuls (0.5 cyc/row) for scores, attnV, and the
K-side projections (K/V feed softmax paths where quantization noise
averages out; the 1/sqrt(dh) scale keeps score errors ~0.5%). Q path and
FFN stay fp16 for the residual/output precision. Host-folded Wkv = Wk@Wv
kills the AllGather; a 2-chunk fp16 AllToAll overlaps the tail with the
second half of attention; softmax exp owns the Act engine.
"""
import sys

import numpy as np

try:
    import concourse.bass  # noqa: F401
except ImportError:
    sys.path.insert(0, "/opt/trn_rl_repo")

import concourse.bacc as bacc
import concourse.mybir as mybir
import concourse.tile as tile
from concourse import bass_utils
from concourse.masks import make_identity

P = 128
S = 2048          # sequence length (Sq == Sk)
D = 1024          # model dim
H = 16            # total heads
DH = 64           # head dim
NCORES = 8
GROUP = 4         # cores per batch group
JC = D // GROUP   # 256 local projection columns
HL = JC // DH     # 4 local heads
DCH = D // P      # 8 d chunks
SCH = S // P      # 16 s chunks
QBP = 1024        # q rows per attention unit (2 units per head)
F32 = mybir.dt.float32
F16 = mybir.dt.float16
BF16 = mybir.dt.bfloat16
F8 = mybir.dt.float8e4
F8E5 = mybir.dt.float8e5
AF = mybir.ActivationFunctionType
OP = mybir.AluOpType
DR = mybir.MatmulPerfMode.DoubleRow
EPS = 1e-5

_CACHE: dict = {}


def _declare_io(nc):
    t = {}
    t["q"] = nc.dram_tensor("q", [S, D], F16, kind="ExternalInput").ap()
    t["k"] = nc.dram_tensor("k", [S, D], F16, kind="ExternalInput").ap()
    t["wq"] = nc.dram_tensor("wq", [D, JC], F16, kind="ExternalInput").ap()
    t["wk"] = nc.dram_tensor("wk", [D, JC], F16, kind="ExternalInput").ap()
    t["wkv"] = nc.dram_tensor("wkv", [D, JC], F8, kind="ExternalInput").ap()
    for b in ("bqp", "bkp"):
        t[b] = nc.dram_tensor(b, [P, 2], F32, kind="ExternalInput").ap()
    t["bvv"] = nc.dram_tensor("bvv", [1, JC], F32, kind="ExternalInput").ap()
    t["wo"] = nc.dram_tensor("wo", [D, D], F16, kind="ExternalInput").ap()
    for b in ("bo", "g0", "b0", "g1", "b1"):
        t[b] = nc.dram_tensor(b, [1, D], F32, kind="ExternalInput").ap()
    t["out"] = nc.dram_tensor("out", [4 * P, D], F32, kind="ExternalOutput").ap()
    return t


def _transpose_in(nc, x_dram, xt, xt_f8=None, chunks=range(4)):
    """x [S, D] f16 DRAM -> xt [128, DCH, S] f16 via the XBAR DMA transpose
    (14ns/16x128 tile, zero PE cost); optional fp8 cast per 512-row chunk."""
    for ch in chunks:
        csl = slice(ch * 512, (ch + 1) * 512)
        nc.sync.dma_start_transpose(xt[:, :, csl], x_dram[csl, :])
        if xt_f8 is not None:
            if ch % 2 == 0:
                nc.vector.tensor_copy(xt_f8[:, :, csl], xt[:, :, csl])
            else:
                nc.scalar.copy(xt_f8[:, :, csl], xt[:, :, csl])


def _emit(nc, tc, ctx, t):
    pools = {}
    pools["const"] = ctx.enter_context(tc.tile_pool(name="const", bufs=1))
    persist_cm = tc.tile_pool(name="persist", bufs=1)
    persist = persist_cm.__enter__()
    dram = ctx.enter_context(tc.tile_pool(name="dram", bufs=1, space="DRAM"))
    pools["pst"] = ctx.enter_context(tc.tile_pool(name="pst", bufs=2, space="PSUM"))
    pools["ps2"] = ctx.enter_context(tc.tile_pool(name="ps2", bufs=2, space="PSUM"))
    ps_a = ctx.enter_context(tc.tile_pool(name="ps_a", bufs=1, space="PSUM"))
    const = pools["const"]

    # constants
    ident = const.tile([P, P], F16)
    make_identity(nc, ident)
    pools["ident"] = ident
    eps_t = const.tile([P, 1], F32)
    nc.vector.memset(eps_t, EPS)
    neg3 = const.tile([P, 1], F32)
    nc.vector.memset(neg3, -3.0)
    pools["neg3"] = neg3
    bqp = const.tile([P, 2], F32)
    bkp = const.tile([P, 2], F32)
    bvb = const.tile([P, JC], F32)
    bob = const.tile([P, D], F32)
    g0b = const.tile([P, D], F32)
    b0b = const.tile([P, D], F32)
    g1b = const.tile([P, D], F32)
    b1b = const.tile([P, D], F32)

    # persistent tiles (q_heads/oh split per q-half to avoid false
    # inter-half dependencies between projection writes and unit reads)
    k_heads = persist.tile([DH, HL, S], F16)
    q_heads = [persist.tile([DH, HL, QBP], F16, name=f"qh{i}")
               for i in range(2)]
    # vp8: [k%128, kc//2, kc%2 slab, head, dh+ones+zeros] fp8, M=128 for
    # DoubleRow attnV (stationary free per slab must be 64 or 128)
    vp8 = persist.tile([P, SCH // 2, 2, HL, P], F8)
    oh = [persist.tile([DH, HL, QBP], F16, name=f"oh{i}") for i in range(2)]

    a2a_in = [dram.tile([QBP, JC], F16, name=f"a2a_in{i}") for i in range(2)]
    a2a_out = [dram.tile([QBP, JC], F16, name=f"a2a_out{i}") for i in range(2)]

    # ---- K path: transposes (f16) -> KpT (fp16, precision) -> k_heads;
    # xt8 = fp8 cast of xt feeds the Vp DoubleRow ----
    w_cm = tc.tile_pool(name="w", bufs=1)
    wpool = w_cm.__enter__()
    wk_sb = wpool.tile([P, DCH, JC], F16)
    wkv8 = wpool.tile([P, DCH, JC], F8)
    wq_sb = wpool.tile([P, DCH, JC], F16)
    xt_cm = tc.tile_pool(name="xt", bufs=1)
    xtp = xt_cm.__enter__()
    xtk = xtp.tile([P, DCH, S], F16)
    xt8 = xtp.tile([P, DCH, S], F8)
    _transpose_in(nc, t["k"], xtk, xt_f8=xt8, chunks=range(2))
    nc.gpsimd.dma_start(wk_sb[:], t["wk"].rearrange("(c p) j -> p c j", p=P))
    nc.gpsimd.dma_start(wkv8[:], t["wkv"].rearrange("(c p) j -> p c j", p=P))
    nc.gpsimd.dma_start(wq_sb[:], t["wq"].rearrange("(c p) j -> p c j", p=P))
    nc.gpsimd.dma_start(bqp[:], t["bqp"])
    nc.gpsimd.dma_start(bkp[:], t["bkp"])
    nc.gpsimd.dma_start(bvb[:], t["bvv"].to_broadcast([P, JC]))
    _transpose_in(nc, t["k"], xtk, xt_f8=xt8, chunks=range(2, 4))
    xt8v = xt8.rearrange("p (g two) s -> p g two s", two=2)
    wkv8v = wkv8.rearrange("p (g two) j -> p g two j", two=2)
    for sh in range(2):
        for jc2 in range(2):
            ps = pools["ps2"].tile([P, QBP], F32, tag="ps2")
            for nb in range(2):
                for dc in range(DCH):
                    off = sh * QBP + nb * 512
                    nc.tensor.matmul(
                        ps[:, nb * 512:(nb + 1) * 512],
                        wk_sb[:, dc, jc2 * P:(jc2 + 1) * P],
                        xtk[:, dc, off:off + 512],
                        start=(dc == 0), stop=(dc == DCH - 1))
            ssl = slice(sh * QBP, (sh + 1) * QBP)
            for hh in range(2):
                rsl = slice(hh * DH, (hh + 1) * DH)
                dst = k_heads[:, 2 * jc2 + hh, ssl]
                if hh == 0:
                    nc.scalar.activation(
                        dst, ps[rsl, :], AF.Identity,
                        bias=bkp[rsl, jc2:jc2 + 1], scale=1.0)
                else:
                    nc.vector.tensor_scalar(
                        out=dst, in0=ps[rsl, :],
                        scalar1=bkp[rsl, jc2:jc2 + 1], scalar2=None, op0=OP.add)
    # Vp natural [S, JC] = K @ (Wk Wv) via fp8 DoubleRow, + ones column
    nc.gpsimd.memset(vp8[:, :, :, :, DH + 1:], 0.0)
    nc.gpsimd.memset(vp8[:, :, :, :, DH:DH + 1], 1.0)
    for sc in range(SCH):
        psv = pools["ps2"].tile([P, QBP], F32, tag="ps2")
        for dcp in range(4):
            nc.tensor.matmul(
                psv[:, 0:JC], xt8v[:, dcp, :, sc * P:(sc + 1) * P],
                wkv8v[:, dcp, :, :], start=(dcp == 0), stop=(dcp == 3),
                perf_mode=DR)
        nc.vector.tensor_tensor(
            out=vp8[:, sc // 2, sc % 2, :, 0:DH],
            in0=psv[:, 0:JC].rearrange("p (h d) -> p h d", h=HL),
            in1=bvb.rearrange("p (h d) -> p h d", h=HL), op=OP.add)
    xt_cm.__exit__(None, None, None)

    # ---- attention/tail pools open before the Q-path xt pools so the
    # xt pools can close (LIFO) mid-attention and release their 38KB ----
    att_cm = tc.tile_pool(name="att", bufs=2)
    att = att_cm.__enter__()
    epool_cm = tc.tile_pool(name="epool", bufs=4)
    epool = epool_cm.__enter__()
    tail_cm = tc.tile_pool(name="tail", bufs=1)
    tailp = tail_cm.__enter__()
    wo_sb = tailp.tile([P, DCH, D], F16)
    nc.gpsimd.dma_start(wo_sb[:], t["wo"].rearrange("(c p) j -> p c j", p=P))
    nc.gpsimd.dma_start(bob[:], t["bo"].to_broadcast([P, D]))
    nc.gpsimd.dma_start(g0b[:], t["g0"].to_broadcast([P, D]))
    nc.gpsimd.dma_start(b0b[:], t["b0"].to_broadcast([P, D]))
    nc.gpsimd.dma_start(g1b[:], t["g1"].to_broadcast([P, D]))
    nc.gpsimd.dma_start(b1b[:], t["b1"].to_broadcast([P, D]))

    # ---- Q path: transposes (fp16) ----
    xt_cm2 = tc.tile_pool(name="xt2", bufs=1)
    xtp2 = xt_cm2.__enter__()
    xtq = xtp2.tile([P, DCH, S], F16)
    _transpose_in(nc, t["q"], xtq, chunks=range(2))

    def q_proj(jc2, shs=(0, 1)):
        """QpT for head pair jc2 -> q_heads (f16) + q8 (fp8)."""
        for sh in shs:
            ps = pools["ps2"].tile([P, QBP], F32, tag="ps2")
            for nb in range(2):
                for dc in range(DCH):
                    off = sh * QBP + nb * 512
                    nc.tensor.matmul(
                        ps[:, nb * 512:(nb + 1) * 512],
                        wq_sb[:, dc, jc2 * P:(jc2 + 1) * P],
                        xtq[:, dc, off:off + 512],
                        start=(dc == 0), stop=(dc == DCH - 1))
            for hh in range(2):
                h = 2 * jc2 + hh
                rsl = slice(hh * DH, (hh + 1) * DH)
                if hh == 0:
                    nc.scalar.activation(
                        q_heads[sh][:, h, :], ps[rsl, :], AF.Identity,
                        bias=bqp[rsl, jc2:jc2 + 1], scale=1.0)
                else:
                    nc.vector.tensor_scalar(
                        out=q_heads[sh][:, h, :], in0=ps[rsl, :],
                        scalar1=bqp[rsl, jc2:jc2 + 1], scalar2=None, op0=OP.add)

    # ---- attention + chunked AllToAll + tail ----

    def att_unit(h, qbp):
        qh = q_heads[qbp]
        psA = ps_a.tile([P, QBP], F32, tag="psA")
        e2s = [None] * (SCH // 2)

        def attnv(kcp):
            for nb in range(2):
                nc.tensor.matmul(
                    psA[:, nb * 512:(nb + 1) * 512],
                    vp8[:, kcp, :, h, :], e2s[kcp][:, :, nb, :],
                    start=(kcp == 0), stop=(kcp == SCH // 2 - 1), perf_mode=DR)

        for kcp in range(SCH // 2):
            e2 = epool.tile([P, 2, 2, 512], F8E5, tag="e")  # [k, slab, nb, q]
            e2s[kcp] = e2
            for i in range(2):
                kc = 2 * kcp + i
                pss = pools["ps2"].tile([P, QBP], F32, tag="ps2")
                for nb in range(2):
                    nc.tensor.matmul(
                        pss[:, nb * 512:(nb + 1) * 512],
                        k_heads[:, h, kc * P:(kc + 1) * P],
                        qh[:, h, nb * 512:(nb + 1) * 512],
                        start=True, stop=True)
                # exp shift -3 keeps e in fp8 range; cancels in the softmax
                nc.scalar.activation(
                    e2[:, i, :, :].rearrange("p a b -> p (a b)"), pss[:],
                    AF.Exp, scale=0.125, bias=pools["neg3"][:])
            # attnV runs one kcp behind so the PE never waits on this
            # chunk's exps (Act) mid-pipeline
            if kcp >= 1:
                attnv(kcp - 1)
        attnv(SCH // 2 - 1)
        # copy numerator+denominator out fast so psA frees for the next unit;
        # reciprocal/normalize pipelined in 512-column halves
        acopy = att.tile([DH + 1, QBP], F32, tag="acopy")
        nc.vector.tensor_copy(acopy[:], psA[0:DH + 1, :])
        recip = att.tile([1, QBP], F32, tag="recip")
        recipb = att.tile([DH, QBP], F32, tag="recipb")
        for hb in range(2):
            hsl = slice(hb * 512, (hb + 1) * 512)
            nc.vector.reciprocal(recip[:, hsl], acopy[DH:DH + 1, hsl])
            nc.gpsimd.partition_broadcast(recipb[:, hsl], recip[:, hsl],
                                          channels=DH)
            nc.vector.tensor_tensor(out=oh[qbp][:, h, hsl],
                                    in0=acopy[0:DH, hsl],
                                    in1=recipb[:, hsl], op=OP.mult)
            nc.vector.tensor_tensor(out=oh[qbp][:, h, hsl],
                                    in0=oh[qbp][:, h, hsl],
                                    in1=qh[:, h, hsl], op=OP.add)

    stgs = {}

    def a2a_head(qbp, h):
        # transpose one head's oh into the staging rows as soon as its unit
        # completes, so only head 3's transposes trail the last unit
        if h == 0:
            stgs[qbp] = [att.tile([P, JC], F16, tag=f"stg{i}", name=f"stg{qbp}_{i}")
                         for i in range(QBP // P)]
        for scq in range(QBP // P):
            pstt = pools["pst"].tile([P, 4 * P], F16, tag="pst")
            nc.tensor.transpose(
                pstt[:, 0:DH],
                oh[qbp][:, h, scq * P:(scq + 1) * P],
                ident[0:DH, 0:DH])
            dst = stgs[qbp][scq][:, h * DH:(h + 1) * DH]
            if scq % 2 == 0:
                nc.vector.tensor_copy(dst, pstt[:, 0:DH])
            else:
                nc.scalar.copy(dst, pstt[:, 0:DH])

    def a2a_chunk(qbp):
        for scq in range(QBP // P):
            nc.gpsimd.dma_start(a2a_in[qbp][scq * P:(scq + 1) * P, :],
                                stgs[qbp][scq][:])
        nc.gpsimd.collective_compute(
            "AllToAll", OP.bypass, ins=[a2a_in[qbp].opt()],
            outs=[a2a_out[qbp].opt()], replica_groups=[list(range(NCORES))])

    def layernorm(tp, sfx, src_ap, dst_ap, gb, bb):
        # E[x], E[x^2] computed concurrently; std = sqrt(ssq/D + (eps - mu^2))
        red = tp.tile([P, 1], F32, tag="red" + sfx)
        nc.vector.tensor_reduce(red[:], src_ap, mybir.AxisListType.X, OP.add)
        negmean = tp.tile([P, 1], F32, tag="negmean" + sfx)
        nc.vector.tensor_scalar_mul(negmean[:], red[:], -1.0 / D)
        sq = tp.tile([P, D], F32, tag="scratchA" + sfx)
        ssq = tp.tile([P, 1], F32, tag="sumsq" + sfx)
        nc.scalar.activation(sq[:], src_ap, AF.Square, bias=negmean[:],
                             scale=1.0, accum_out=ssq[:])
        std = tp.tile([P, 1], F32, tag="std" + sfx)
        nc.scalar.activation(std[:], ssq[:], AF.Sqrt, bias=eps_t[:],
                             scale=1.0 / D)
        rstd = tp.tile([P, 1], F32, tag="rstd" + sfx)
        nc.vector.reciprocal(rstd[:], std[:])
        nc.vector.tensor_scalar(out=dst_ap, in0=src_ap, scalar1=negmean[:],
                                scalar2=rstd[:], op0=OP.add, op1=OP.mult)
        nc.vector.tensor_tensor(out=dst_ap, in0=dst_ap, in1=gb[:], op=OP.mult)
        nc.vector.tensor_tensor(out=dst_ap, in0=dst_ap, in1=bb[:], op=OP.add)

    def tail_stage1(qbp, b2, tp, sfx):
        osb = tp.tile([P, D], F16, tag="osb" + sfx)
        for j in range(GROUP):
            nc.sync.dma_start(
                osb[:, j * JC:(j + 1) * JC],
                a2a_out[qbp][(GROUP * b2 + j) * P:(GROUP * b2 + j + 1) * P, :])
        ln0 = tp.tile([P, D], F16, tag="ln0" + sfx)
        layernorm(tp, sfx, osb[:], ln0[:], g0b, b0b)
        ln0t = tp.tile([P, DCH, P], F16, tag="ln0t" + sfx)
        for dcg in range(2):
            pstt = pools["pst"].tile([P, 4 * P], F16, tag="pst")
            for i in range(4):
                dc = 4 * dcg + i
                nc.tensor.transpose(pstt[:, i * P:(i + 1) * P],
                                    ln0[:, dc * P:(dc + 1) * P], ident)
            nc.vector.tensor_copy(
                ln0t[:, 4 * dcg:4 * dcg + 4, :],
                pstt.rearrange("p (c q) -> p c q", c=4))
        return ln0, ln0t

    def tail_stage2(qbp, b2, tp, sfx, ln0, ln0t):
        pso = pools["ps2"].tile([P, QBP], F32, tag="ps2")
        for dc in range(DCH):
            for nb in range(2):
                nc.tensor.matmul(
                    pso[:, nb * 512:(nb + 1) * 512], ln0t[:, dc, :],
                    wo_sb[:, dc, nb * 512:(nb + 1) * 512],
                    start=(dc == 0), stop=(dc == DCH - 1))
        fb = tp.tile([P, D], F32, tag="scratchA" + sfx)
        nc.vector.tensor_tensor(out=fb[:], in0=pso[:], in1=bob[:], op=OP.add)
        gel = tp.tile([P, D], F32, tag="gel" + sfx)
        nc.scalar.activation(gel[:], fb[:], AF.Gelu)
        o2 = tp.tile([P, D], F32, tag="o2" + sfx)
        nc.vector.tensor_tensor(out=o2[:], in0=ln0[:], in1=gel[:], op=OP.add)
        fin = tp.tile([P, D], F32, tag="fin" + sfx)
        layernorm(tp, sfx, o2[:], fin[:], g1b, b1b)
        nc.gpsimd.dma_start(
            t["out"][(2 * qbp + b2) * P:(2 * qbp + b2 + 1) * P, :], fin[:])

    def tail_block(qbp, b2, tp=None, sfx=""):
        tp = tp or tailp
        ln0, ln0t = tail_stage1(qbp, b2, tp, sfx)
        tail_stage2(qbp, b2, tp, sfx, ln0, ln0t)

    # interleaved emission: overlap Q projections with early attention,
    # and chunk-0 tail with the AllToAlls
    q_proj(0, shs=(0,))
    q_proj(1, shs=(0,))
    att_unit(0, 0)
    a2a_head(0, 0)
    _transpose_in(nc, t["q"], xtq, chunks=range(2, 4))
    att_unit(1, 0)
    a2a_head(0, 1)
    q_proj(0, shs=(1,))
    q_proj(1, shs=(1,))
    att_unit(2, 0)
    a2a_head(0, 2)
    att_unit(3, 0)
    a2a_head(0, 3)
    xt_cm2.__exit__(None, None, None)
    a2a_chunk(0)
    att_unit(0, 1)
    a2a_head(1, 0)
    att_unit(1, 1)
    a2a_head(1, 1)
    att_unit(2, 1)
    a2a_head(1, 2)
    att_unit(3, 1)
    a2a_head(1, 3)
    tail_block(0, 0)
    a2a_chunk(1)
    tail2_cm = tc.tile_pool(name="tail2", bufs=1)
    tail2p = tail2_cm.__enter__()
    tail_block(0, 1, tp=tail2p, sfx="b")
    s1a = tail_stage1(1, 0, tailp, "")
    s1b = tail_stage1(1, 1, tail2p, "b")
    tail_stage2(1, 0, tailp, "", *s1a)
    tail_stage2(1, 1, tail2p, "b", *s1b)

    tail2_cm.__exit__(None, None, None)
    tail_cm.__exit__(None, None, None)
    epool_cm.__exit__(None, None, None)
    att_cm.__exit__(None, None, None)
    w_cm.__exit__(None, None, None)
    persist_cm.__exit__(None, None, None)


def build():
    if "nc" in _CACHE:
        return _CACHE["nc"]
    from contextlib import ExitStack
    nc = bacc.Bacc("TRN2", target_bir_lowering=False, debug=False,
                   num_devices=NCORES)
    t = _declare_io(nc)
    with tile.TileContext(nc) as tc:
        with ExitStack() as ctx:
            _emit(nc, tc, ctx, t)
    nc.compile()
    _CACHE["nc"] = nc
    return nc


def make_in_maps(Q, K, Wq, bq, Wk, bk, Wv, bv, Wo, bo, g0, b0, g1, b1):
    import ml_dtypes
    f16 = np.float16
    f32 = np.float32
    f8 = ml_dtypes.float8_e4m3
    Wkv = (Wk.astype(f32) @ Wv.astype(f32))
    bkv = (bk.astype(f32) @ Wv.astype(f32) + bv.astype(f32))
    Qh = [np.ascontiguousarray(Q[b].astype(f16)) for b in range(2)]
    Kh = [np.ascontiguousarray(K[b].astype(f16)) for b in range(2)]
    Wo16 = np.ascontiguousarray(Wo.astype(f16))
    in_maps = []
    for c in range(NCORES):
        b, g = divmod(c, GROUP)
        jsl = slice(g * JC, (g + 1) * JC)
        ac = np.ascontiguousarray
        in_maps.append({
            "q": Qh[b], "k": Kh[b],
            "wq": ac(Wq[:, jsl].astype(f16)),
            "wk": ac(Wk[:, jsl].astype(f16)),
            "wkv": ac(Wkv[:, jsl].astype(f8)),
            "bqp": ac(bq[jsl].astype(f32).reshape(2, P).T),
            "bkp": ac(bk[jsl].astype(f32).reshape(2, P).T),
            "bvv": ac(bkv[jsl].reshape(1, JC)),
            "wo": Wo16, "bo": ac(bo.astype(f32).reshape(1, D)),
            "g0": ac(g0.astype(f32).reshape(1, D)),
            "b0": ac(b0.astype(f32).reshape(1, D)),
            "g1": ac(g1.astype(f32).reshape(1, D)),
            "b1": ac(b1.astype(f32).reshape(1, D)),
        })
    return in_maps


def run(in_maps, trace=False, **kwargs):
    nc = build()
    return bass_utils.run_bass_kernel_spmd(
        nc, in_maps, core_ids=list(range(NCORES)), trace=trace, **kwargs)


def kernel(**inputs):
    inputs = {k: np.asarray(v) for k, v in inputs.items()}
    in_maps = make_in_maps(
        inputs["Q"], inputs["K"], inputs["Wq"], inputs["bq"], inputs["Wk"],
        inputs["bk"], inputs["Wv"], inputs["bv"], inputs["Wo"], inputs["bo"],
        inputs["g0"], inputs["b0"], inputs["g1"], inputs["b1"])
    res = run(in_maps, trace=False)
    out = np.empty((2, S, D), dtype=np.float32)
    for c in range(NCORES):
        r = res.results[c]["out"]  # [512, D] blocks: (qbp0,b0),(qbp0,b1),(qbp1,b0),(qbp1,b1)
        for qbp in range(2):
            for b in range(2):
                out[b, qbp * QBP + c * P:qbp * QBP + (c + 1) * P, :] = \
                    r[(2 * qbp + b) * P:(2 * qbp + b + 1) * P]
    return out


if __name__ == "__main__":
    rng = np.random.default_rng(0)
    ins = {n: rng.standard_normal(s).astype(np.float32) * (0.03125 if n.startswith("W") else 1.0)
           for n, s in [("Q", (2, S, D)), ("K", (2, S, D)), ("Wq", (D, D)),
                        ("Wk", (D, D)), ("Wv", (D, D)), ("Wo", (D, D))]}
    for n in ("bq", "bk", "bv", "bo", "b0", "b1"):
        ins[n] = np.zeros(D, np.float32)
    for n in ("g0", "g1"):
        ins[n] = np.ones(D, np.float32)
    out = kernel(**ins)
    print("ran ok", out.shape, out.dtype)
